# revision 7
# baseline (speedup 1.0000x reference)
"""Causal self-attention (GQA + RoPE) Trainium2 Bass kernel, 8-way sharded.

Sharding: core c -> batch b = c // 2, head-half hh = c % 2.
Each core computes the qkv projection, attention and output projection for
its batch and its 16 query heads / 4 kv heads (kv groups kept whole); the
output projection is a row-shard of Wproj, so the two cores of a batch
produce partial sums that the host adds.

Device-side layout tricks (host prepares):
  - x is fed pre-transposed (xT [C, T]) so the qkv matmul needs no on-device
    transpose of x.
  - Wq/Wk columns are de-interleaved per head (even rope pairs then odd), so
    RoPE becomes the rotate-half form with free-dim slices only.
  - scores are computed transposed (scoresT = k_tile^T-matmul) so the
    attention-weights matmul needs no transposes; softmax normalization is
    reconstructed via a ones-row matmul + reciprocal + PE broadcast.

All PE matmuls run in float32r (full rate for moving dim >= 256).
"""

import os

os.environ.setdefault("JAX_PLATFORMS", "axon")

import numpy as np

B, T, C = 4, 1024, 4096
H, KV, HD = 32, 8, 128
REP = H // KV  # 4

NQ = 16      # q heads per core
NKV = 4      # kv heads per core
QK_HEADS = NQ + NKV          # 20 rope'd/transposed heads
COLS = (NQ + 2 * NKV) * HD   # 3072 local qkv cols: q0..15 k0..3 v0..3
NTT = T // 128               # 8 token tiles
SCALE = float(1.0 / np.sqrt(np.float32(HD)).astype(np.float32))

_CACHE: dict = {}


def _build_nc():
    import concourse.mybir as mybir
    import concourse.tile as tile
    from concourse import bacc
    from concourse.bass import ts
    from concourse.masks import make_identity

    f32 = mybir.dt.float32
    f32r = mybir.dt.float32r
    Exp = mybir.ActivationFunctionType.Exp

    nc = bacc.Bacc(None, target_bir_lowering=False, debug=False)

    xT_d = nc.dram_tensor("xT", [C, T], f32r, kind="ExternalInput")
    # [j(256-col chunk), cc(128-row chunk), 128, 256]
    wqkv_d = nc.dram_tensor("wqkv", [12, 32, 128, 256], f32r, kind="ExternalInput")
    # [ccol(512-col chunk), ycc(128-row chunk), 128, 512]
    wproj_d = nc.dram_tensor("wproj", [8, 16, 128, 512], f32r, kind="ExternalInput")
    cos_d = nc.dram_tensor("cosn", [T, 64], f32, kind="ExternalInput")
    sin_d = nc.dram_tensor("sinn", [T, 64], f32, kind="ExternalInput")
    # mask_rel[d][p][f] = 1.0 if 128*d + p <= f else 0.0
    masks_d = nc.dram_tensor("masks", [4, 128, 512], f32, kind="ExternalInput")
    out_d = nc.dram_tensor("out", [T, C], f32, kind="ExternalOutput")
    # scratch: q/k transposed [head, hd=128, T]; v natural [T, 512]
    qkT_d = nc.dram_tensor("qkT_scratch", [QK_HEADS, 128, T], f32r)
    v_d = nc.dram_tensor("v_scratch", [T, NKV * HD], f32r)

    with tile.TileContext(nc) as tc, tc.tile_pool(
        name="const", bufs=1
    ) as const_p, tc.tile_pool(name="psA", bufs=3, space="PSUM") as psA:
        ident = const_p.tile([128, 128], f32)
        make_identity(nc, ident[:])
        ones0 = const_p.tile([128, 1], f32)
        nc.vector.memset(ones0[:], 1.0)
        ones_red = const_p.tile([128, 1], f32r)
        nc.scalar.copy(out=ones_red[:], in_=ones0[:])
        ones_row0 = const_p.tile([1, 128], f32)
        nc.vector.memset(ones_row0[:], 1.0)
        ones_row = const_p.tile([1, 128], f32r)
        nc.scalar.copy(out=ones_row[:], in_=ones_row0[:])
        cos_sb = const_p.tile([128, NTT, 64], f32)
        nc.sync.dma_start(
            out=cos_sb[:], in_=cos_d.rearrange("(tt p) j -> p tt j", p=128)
        )
        sin_sb = const_p.tile([128, NTT, 64], f32)
        nc.sync.dma_start(
            out=sin_sb[:], in_=sin_d.rearrange("(tt p) j -> p tt j", p=128)
        )

        # ================= PHASE 1: qkv = x @ Wqkv (+RoPE, +transposes) =====
        xT_r = xT_d.rearrange("(cc p) t -> p cc t", p=128)  # [128, 32, 1024]
        with (
            tc.tile_pool(name="x", bufs=1) as x_p,
            tc.tile_pool(name="w", bufs=2) as w_p,
            tc.tile_pool(name="rope", bufs=2) as rope_p,
            tc.tile_pool(name="tstage", bufs=4) as tstage_p,
            tc.tile_pool(name="vstage", bufs=2) as vstage_p,
            tc.tile_pool(name="psT", bufs=2, space="PSUM") as psT,
        ):
            for th in range(2):  # T halves
                xh = x_p.tile([128, 32, 512], f32r, tag="x")
                nc.sync.dma_start(out=xh[:], in_=xT_r[:, :, ts(th, 512)])
                for j in range(12):
                    wt = w_p.tile([128, 32, 256], f32r, tag="w")
                    nc.sync.dma_start(
                        out=wt[:], in_=wqkv_d[j].rearrange("cc p f -> p cc f")
                    )
                    for tt in range(4):
                        tg = th * 4 + tt  # global token tile
                        ps = psA.tile([128, 256], f32, tag="psA")
                        for cc in range(32):
                            nc.tensor.matmul(
                                ps[:],
                                xh[:, cc, ts(tt, 128)],
                                wt[:, cc, :],
                                start=(cc == 0),
                                stop=(cc == 31),
                            )
                        if j < 10:
                            # two rope'd heads per 256-col chunk
                            a = ps[:].rearrange("p (h x j) -> p h x j", x=2, j=64)
                            cosb = cos_sb[:, tg, :].unsqueeze(1).broadcast_to([128, 2, 64])
                            sinb = sin_sb[:, tg, :].unsqueeze(1).broadcast_to([128, 2, 64])
                            rt = rope_p.tile([128, 2, 2, 64], f32, tag="rt")
                            t0 = rope_p.tile([128, 2, 64], f32, tag="t0")
                            t1 = rope_p.tile([128, 2, 64], f32, tag="t1")
                            nc.vector.tensor_mul(t0[:], a[:, :, 0, :], cosb)
                            nc.vector.tensor_mul(t1[:], a[:, :, 1, :], sinb)
                            nc.vector.tensor_sub(rt[:, :, 0, :], t0[:], t1[:])
                            t2 = rope_p.tile([128, 2, 64], f32, tag="t0")
                            t3 = rope_p.tile([128, 2, 64], f32, tag="t1")
                            nc.vector.tensor_mul(t2[:], a[:, :, 1, :], cosb)
                            nc.vector.tensor_mul(t3[:], a[:, :, 0, :], sinb)
                            nc.vector.tensor_add(rt[:, :, 1, :], t2[:], t3[:])
                            for hh in range(2):
                                h = 2 * j + hh
                                pt = psT.tile([128, 128], f32, tag="psT")
                                nc.tensor.transpose(
                                    pt[:],
                                    rt[:, hh, :, :].rearrange("p x j -> p (x j)"),
                                    ident[:],
                                )
                                st = tstage_p.tile([128, 128], f32r, tag="ts")
                                nc.scalar.copy(out=st[:], in_=pt[:])
                                nc.sync.dma_start(
                                    out=qkT_d[h, :, ts(tg, 128)], in_=st[:]
                                )
                        else:
                            vs = vstage_p.tile([128, 256], f32r, tag="vs")
                            nc.scalar.copy(out=vs[:], in_=ps[:])
                            nc.sync.dma_start(
                                out=v_d[ts(tg, 128), ts(j - 10, 256)], in_=vs[:]
                            )

        # ================= PHASE 2: attention ==============================
        with (
            tc.tile_pool(name="yt", bufs=NQ) as yt_p,
            tc.tile_pool(name="vsb", bufs=1) as vsb_p,
            tc.tile_pool(name="msk", bufs=1) as msk_p,
        ):
            yts = [yt_p.tile([128, T], f32r, tag="yt", name=f"yt{i}") for i in range(NQ)]
            v_sb = vsb_p.tile([128, NTT, NKV * HD], f32r)
            nc.sync.dma_start(
                out=v_sb[:], in_=v_d.rearrange("(tt p) f -> p tt f", p=128)
            )
            mask_sb = msk_p.tile([128, 4, 512], f32)
            nc.sync.dma_start(out=mask_sb[:], in_=masks_d.rearrange("d p f -> p d f"))

            with (
                tc.tile_pool(name="qt", bufs=2) as qt_p,
                tc.tile_pool(name="kt", bufs=2) as kt_p,
                tc.tile_pool(name="exp", bufs=4) as exp_p,
                tc.tile_pool(name="small", bufs=4) as small_p,
                tc.tile_pool(name="psY", bufs=2, space="PSUM") as psY,
                tc.tile_pool(name="psS", bufs=2, space="PSUM") as psS,
            ):
                for g in range(NKV):
                    kt = kt_p.tile([128, T], f32r, tag="kt")
                    nc.sync.dma_start(out=kt[:], in_=qkT_d[NQ + g])
                    for r in range(REP):
                        hq = g * REP + r
                        qt = qt_p.tile([128, T], f32r, tag="qt")
                        nc.sync.dma_start(out=qt[:], in_=qkT_d[hq])
                        for chunk in range(2):
                            tq0 = 512 * chunk
                            ns = 4 * (chunk + 1)
                            py = psY.tile([128, 512], f32, tag="psY")
                            psum = psS.tile([1, 512], f32, tag="psS")
                            for si in range(ns):
                                pss = psA.tile([128, 512], f32, tag="psA")
                                nc.tensor.matmul(
                                    pss[:],
                                    kt[:, ts(si, 128)],
                                    qt[:, tq0 : tq0 + 512],
                                    start=True,
                                    stop=True,
                                )
                                et = exp_p.tile([128, 512], f32r, tag="exp")
                                nc.scalar.activation(
                                    out=et[:], in_=pss[:], func=Exp, scale=SCALE
                                )
                                d = si - 4 * chunk
                                if d >= 0:
                                    nc.vector.tensor_mul(
                                        et[:], et[:], mask_sb[:, d, :]
                                    )
                                nc.tensor.matmul(
                                    py[:],
                                    v_sb[:, si, ts(g, 128)],
                                    et[:],
                                    start=(si == 0),
                                    stop=(si == ns - 1),
                                )
                                nc.tensor.matmul(
                                    psum[:],
                                    ones_red[:],
                                    et[:],
                                    start=(si == 0),
                                    stop=(si == ns - 1),
                                )
                            recip = small_p.tile([1, 512], f32r, tag="recip")
                            with nc.allow_low_precision(reason="fp32r softmax recip"):
                                nc.vector.reciprocal(out=recip[:], in_=psum[:])
                            prb = psS.tile([128, 512], f32, tag="psS")
                            nc.tensor.matmul(
                                prb[:],
                                ones_row[:],
                                recip[:],
                                start=True,
                                stop=True,
                            )
                            rb = small_p.tile([128, 512], f32, tag="rb")
                            nc.scalar.copy(out=rb[:], in_=prb[:])
                            nc.vector.tensor_mul(
                                yts[hq][:, tq0 : tq0 + 512], py[:], rb[:]
                            )

            # ============= PHASE 3: out = y @ Wproj (row shard) ============
            with (
                tc.tile_pool(name="wp", bufs=2) as wp_p,
                tc.tile_pool(name="ostage", bufs=3) as ostage_p,
            ):
                for ccol in range(8):
                    wp = wp_p.tile([128, 16, 512], f32r, tag="wp")
                    nc.sync.dma_start(
                        out=wp[:], in_=wproj_d[ccol].rearrange("y p f -> p y f")
                    )
                    for tt in range(NTT):
                        po = psA.tile([128, 512], f32, tag="psA")
                        for ycc in range(16):
                            nc.tensor.matmul(
                                po[:],
                                yts[ycc][:, ts(tt, 128)],
                                wp[:, ycc, :],
                                start=(ycc == 0),
                                stop=(ycc == 15),
                            )
                        ot = ostage_p.tile([128, 512], f32, tag="os")
                        nc.scalar.copy(out=ot[:], in_=po[:])
                        nc.sync.dma_start(
                            out=out_d[ts(tt, 128), ts(ccol, 512)], in_=ot[:]
                        )

    nc.compile()
    return nc


def prep_inputs(x, Wqkv, Wproj, freqs_cos, freqs_sin):
    """Build the 8 per-core input maps (host-side shard + layout prep)."""
    x = np.asarray(x, np.float32)
    Wqkv = np.asarray(Wqkv, np.float32)
    Wproj = np.asarray(Wproj, np.float32)
    cos = np.ascontiguousarray(np.asarray(freqs_cos, np.float32))
    sin = np.ascontiguousarray(np.asarray(freqs_sin, np.float32))

    perm = np.concatenate([np.arange(0, HD, 2), np.arange(1, HD, 2)])
    masks = (
        (128 * np.arange(4)[:, None, None] + np.arange(128)[None, :, None])
        <= np.arange(512)[None, None, :]
    ).astype(np.float32)
    masks = np.ascontiguousarray(masks)

    in_maps = []
    for c in range(8):
        b, hh = divmod(c, 2)
        qcols = (hh * NQ * HD + (np.arange(NQ) * HD)[:, None] + perm[None, :]).ravel()
        kcols = (
            H * HD + hh * NKV * HD + (np.arange(NKV) * HD)[:, None] + perm[None, :]
        ).ravel()
        vcols = (
            (H + KV) * HD
            + hh * NKV * HD
            + (np.arange(NKV) * HD)[:, None]
            + np.arange(HD)[None, :]
        ).ravel()
        col_idx = np.concatenate([qcols, kcols, vcols])
        Wc = Wqkv[:, col_idx]  # [4096, 3072]
        wq = np.ascontiguousarray(Wc.reshape(32, 128, 12, 256).transpose(2, 0, 1, 3))
        Wp = Wproj[hh * NQ * HD : (hh + 1) * NQ * HD, :]  # [2048, 4096]
        wp = np.ascontiguousarray(Wp.reshape(16, 128, 8, 512).transpose(2, 0, 1, 3))
        xT = np.ascontiguousarray(x[b].T)  # [4096, 1024]
        in_maps.append(
            {"xT": xT, "wqkv": wq, "wproj": wp, "cosn": cos, "sinn": sin,
             "masks": masks}
        )
    return in_maps


def _get_nc():
    if "nc" not in _CACHE:
        _CACHE["nc"] = _build_nc()
    return _CACHE["nc"]


def kernel(x, Wqkv, Wproj, freqs_cos, freqs_sin, mask=None):
    from concourse.bass_utils import run_bass_kernel_spmd

    nc = _get_nc()
    in_maps = prep_inputs(x, Wqkv, Wproj, freqs_cos, freqs_sin)
    res = run_bass_kernel_spmd(nc, in_maps, core_ids=list(range(8)))
    outs = [res.results[c]["out"] for c in range(8)]
    y = np.stack([outs[2 * b] + outs[2 * b + 1] for b in range(B)], axis=0)
    return y.astype(np.float32)
